# revision 3
# baseline (speedup 1.0000x reference)
"""Dense dot-product attention with key-length masking on 8 Trainium2 cores.

Problem: q,k,v [16, 2048, 128] fp32, valid_lens [16,1] int32.
  out = softmax(mask(q@k.T/sqrt(d))) @ v   (masked keys -> -1e6 before softmax)

The kernel is PSUM-drain bound: every score element must leave PSUM through
ScalarE (ACT) or VectorE (DVE) - the only engines that read PSUM.  The design
splits that drain and everything downstream across all five engines:

- S^T tiles (keys on partitions) from PE; fp16 operands, host pre-transposed.
- Fully-valid key tiles: ScalarE exp() with immediate scale (no operand fetch).
- Masked / overflow key tiles: DVE Schraudolph exp - one tensor_scalar
  computes round(S*sc_k + bi_k) into int16 (RNE, verified) whose bitcast IS
  fp16 exp(S/sqrt(d)) to ~3%;  sc_k/bi_k are per-partition vectors, so masked
  keys land exactly on +0.0.  Softmax renormalization cancels the shared
  Schraudolph bias (C chosen to zero the mean vs true exp).
- O^T accumulates over key tiles with V stationary, E moving (fp16).
- Softmax denominators: per-slot running sums of E tiles split between GpSimd
  (slow but otherwise idle) and DVE; host finishes the 128-partition sum and
  the divide + transpose.  oT is copied PSUM->SBUF as fp16 by ScalarE.
- Inputs are packed per slot into one contiguous [128, X] fp16 DRAM buffer
  (qT | kT | v-permuted) -> one dma_start per slot (slot 0 split into pieces
  so the pipeline ignites early); this cuts descriptor-issue cost ~3x.
- HAM warm-up: dummy bf16 matmuls run while the input DMAs stream.

Work distribution (valid_lens-aware, single SPMD program): 32 units
(16 batches x 2 query halves), work = ceil(L/128) key tiles; units sorted by
work into 4 groups of 8 (one unit per core per slot); per-slot trip count
baked as the group max.  Slot order: small group first (input load gates
compute start), smallest last (its drain is the tail).
"""

import math
import sys
import types

import numpy as np

import concourse.bass as bass
import concourse.mybir as mybir
import concourse.tile as tile
from concourse import bacc
from concourse.bass_utils import run_bass_kernel_spmd

B, Q, K, D = 16, 2048, 2048, 128
NCORES = 8
QCH = 1024         # queries per work unit
UNITS = B * (Q // QCH)
NSLOT = UNITS // NCORES
MM_N = 512         # moving-operand free dim per matmul
KT = K // 128      # max key tiles
SCALE = 1.0 / math.sqrt(D)
LOG2E = 1.4426950408889634
SCHC = 0.0574      # Schraudolph shift: zero-mean vs true exp under softmax
WARMUP_MMS = 9     # dummy matmuls to lift the PE HAM clock-gate
GP_FRAC = 0.38     # fraction of the denominator tree on GpSimd

F32 = mybir.dt.float32
F16 = mybir.dt.float16
I16 = mybir.dt.int16
BF16 = mybir.dt.bfloat16


def _install_hook_stub():
    """bass_utils' axon trace path imports antenv.axon_hooks, which is not
    shipped in this container.  Provide a no-op stub so an ambient
    BASS_TRACE=1 doesn't crash; test harnesses may overwrite the hook."""
    if "antenv.axon_hooks" in sys.modules:
        return
    mod = types.ModuleType("antenv.axon_hooks")
    _hook = [None]
    mod.set_axon_ntff_profile_hook = lambda h: _hook.__setitem__(0, h)
    mod.get_axon_ntff_profile_hook = lambda: _hook[0]
    sys.modules["antenv.axon_hooks"] = mod


_install_hook_stub()

_build_cache = {}
last_result = None  # BassKernelResults of the most recent run (for harnesses)


def _tree_split(t, n_act):
    """Assign each of the t E tiles of a slot to the GpSimd or DVE partial
    accumulator, spreading GpSimd's (slow, serial) share across the slot."""
    gp_cnt = min(t - 0, max(0, round(GP_FRAC * t)))
    gp = set()
    acc = 0.0
    for i in range(t):
        acc += gp_cnt / t
        if acc >= 1.0 and len(gp) < gp_cnt:
            acc -= 1.0
            gp.add(i)
    return gp


def _build(trips, nact):
    """One SPMD program: slot j processes trips[j] key tiles of one unit.
    The first nact[j] key tiles are fully valid on every core of the group
    and drain through ScalarE exp() with immediate scale; the rest drain
    through DVE Schraudolph with per-partition scale/bias (data-masked)."""
    nc = bacc.Bacc(num_devices=NCORES)

    t_all = list(trips)
    xlens = [QCH + 2 * 128 * t for t in t_all]
    inbs = [
        nc.declare_dram_parameter(f"inb{s}", [128, xlens[s]], F16, isOutput=False)
        for s in range(NSLOT)
    ]
    scbi = nc.declare_dram_parameter("scbi", [NSLOT, 128, 2 * KT], F32, isOutput=False)
    oT = nc.declare_dram_parameter("oT", [NSLOT, 128, QCH], F16, isOutput=True)
    esum = nc.declare_dram_parameter("esum", [NSLOT, 128, QCH], F16, isOutput=True)

    with tile.TileContext(nc) as tc:
        with (
            tc.tile_pool(name="consts", bufs=1) as consts,
            tc.tile_pool(name="inputs", bufs=2) as inpool,
            tc.tile_pool(name="scp", bufs=2) as scpool,
            tc.tile_pool(name="epool", bufs=max(trips) + 6) as epool,
            tc.tile_pool(name="dvacc", bufs=2) as dvaccp,
            tc.tile_pool(name="gpacc", bufs=2) as gpaccp,
            tc.tile_pool(name="osb", bufs=2) as opool,
            tc.tile_pool(name="sps", bufs=3, space="PSUM") as pspool,
            tc.tile_pool(name="oacc", bufs=1, space="PSUM") as psacc,
        ):
            # --- HAM warm-up: dummy bf16 matmuls while input DMAs stream ---
            wsrc = consts.tile([128, MM_N], BF16)
            nc.vector.memset(wsrc[:], 1.0)
            for w in range(WARMUP_MMS):
                if w % 2 == 0:
                    wps = pspool.tile([128, QCH], F32, tag="s")
                nc.tensor.matmul(
                    wps[:, (w % 2) * MM_N : (w % 2) * MM_N + MM_N],
                    wsrc[:, :128],
                    wsrc[:],
                    start=True,
                    stop=True,
                    skip_group_check=True,
                )

            for s in range(NSLOT):
                t = t_all[s]
                nf = nact[s]
                inb = inpool.tile([128, xlens[s]], F16, tag="inb")
                sc_sb = scpool.tile([128, 2 * KT], F32, tag="scbi")
                kbase, vbase = QCH, QCH + 128 * t
                if s == 0:
                    # ignition pieces on distinct queues: each engine's first
                    # DMA gets an early start, so tile 0 computes ASAP
                    nc.sync.dma_start(out=inb[:, 0:MM_N], in_=inbs[s][:, 0:MM_N])
                    nc.scalar.dma_start(
                        out=inb[:, MM_N:QCH], in_=inbs[s][:, MM_N:QCH]
                    )
                    nc.gpsimd.dma_start(
                        out=inb[:, kbase : kbase + 128],
                        in_=inbs[s][:, kbase : kbase + 128],
                    )
                    nc.gpsimd.dma_start(
                        out=inb[:, vbase : vbase + 128],
                        in_=inbs[s][:, vbase : vbase + 128],
                    )
                    # rest of kT, then rest of v
                    nc.sync.dma_start(
                        out=inb[:, kbase + 128 : vbase],
                        in_=inbs[s][:, kbase + 128 : vbase],
                    )
                    nc.sync.dma_start(
                        out=inb[:, vbase + 128 :], in_=inbs[s][:, vbase + 128 :]
                    )
                    nc.scalar.dma_start(out=sc_sb[:], in_=scbi[s])
                else:
                    nc.sync.dma_start(out=inb[:], in_=inbs[s][:])
                    nc.gpsimd.dma_start(out=sc_sb[:], in_=scbi[s])

                gp_set = _tree_split(t, nf)
                o_ps = psacc.tile([128, QCH], F32, tag="o")
                acc_v = None  # DVE partial-sum chain head (an E tile AP)
                acc_g = None  # GpSimd chain head
                n_v = n_g = 0
                dv_tile = gp_tile = None
                for i in range(t):
                    s_ps = pspool.tile([128, QCH], F32, tag="s")
                    for h in range(QCH // MM_N):
                        nc.tensor.matmul(
                            s_ps[:, bass.ts(h, MM_N)],
                            inb[:, kbase + i * 128 : kbase + (i + 1) * 128],
                            inb[:, bass.ts(h, MM_N)],
                            start=True,
                            stop=True,
                        )
                    if i < nf:
                        e_sb = epool.tile([128, QCH], F16, tag="e")
                        e_ap = e_sb[:]
                        nc.scalar.activation(
                            e_ap,
                            s_ps[:],
                            mybir.ActivationFunctionType.Exp,
                            scale=float(SCALE),
                        )
                    else:
                        e16 = epool.tile([128, QCH], I16, tag="e")
                        nc.vector.tensor_scalar(
                            e16[:],
                            s_ps[:],
                            sc_sb[:, i : i + 1],
                            sc_sb[:, KT + i : KT + i + 1],
                            mybir.AluOpType.mult,
                            mybir.AluOpType.add,
                        )
                        e_ap = e16[:].bitcast(F16)
                    for h in range(QCH // MM_N):
                        nc.tensor.matmul(
                            o_ps[:, bass.ts(h, MM_N)],
                            inb[:, vbase + i * 128 : vbase + (i + 1) * 128],
                            e_ap[:, bass.ts(h, MM_N)],
                            start=(i == 0),
                            stop=(i == t - 1),
                        )
                    # denominator partial sums: two independent chains
                    if i in gp_set:
                        if acc_g is None:
                            acc_g, n_g = e_ap, 1
                        elif n_g == 1:
                            gp_tile = gpaccp.tile([128, QCH], F16, tag="ga")
                            nc.gpsimd.tensor_add(gp_tile[:], acc_g, e_ap)
                            acc_g, n_g = gp_tile[:], 2
                        else:
                            nc.gpsimd.tensor_add(acc_g, acc_g, e_ap)
                    else:
                        if acc_v is None:
                            acc_v, n_v = e_ap, 1
                        elif n_v == 1:
                            dv_tile = dvaccp.tile([128, QCH], F16, tag="va")
                            nc.vector.tensor_add(dv_tile[:], acc_v, e_ap)
                            acc_v, n_v = dv_tile[:], 2
                        else:
                            nc.vector.tensor_add(acc_v, acc_v, e_ap)

                # combine the two chains and ship the denominators
                if acc_g is None:
                    es_ap = acc_v
                elif acc_v is None:
                    es_ap = acc_g
                else:
                    if n_v == 1:  # chain head still lives in the E pool
                        dv_tile = dvaccp.tile([128, QCH], F16, tag="va")
                        nc.vector.tensor_add(dv_tile[:], acc_v, acc_g)
                        es_ap = dv_tile[:]
                    else:
                        nc.vector.tensor_add(acc_v, acc_v, acc_g)
                        es_ap = acc_v
                nc.sync.dma_start(out=esum[s], in_=es_ap)

                o_sb = opool.tile([128, QCH], F16, tag="osb")
                nc.scalar.copy(o_sb[:], o_ps[:])
                nc.sync.dma_start(out=oT[s], in_=o_sb[:])

    nc.compile()
    return nc


def kernel(q, k, v, valid_lens):
    q = np.ascontiguousarray(q, dtype=np.float32)
    k = np.ascontiguousarray(k, dtype=np.float32)
    v = np.ascontiguousarray(v, dtype=np.float32)
    L = np.asarray(valid_lens).reshape(-1).astype(np.int64)

    # per-batch key-tile need; L==0 batches are handled entirely on the host
    # (uniform softmax over all keys == plain mean of v)
    need = np.minimum(KT, (L + 127) // 128).astype(np.int64)

    units = [(int(need[b]), b, h) for b in range(B) for h in range(Q // QCH)]
    units.sort(key=lambda u: u[0])
    group_order = [1, NSLOT - 1] + list(range(NSLOT - 2, 1, -1)) + [0]
    trips = tuple(
        max(1, units[NCORES * g + NCORES - 1][0]) for g in group_order
    )
    # leading key tiles fully valid on every core of the slot go to ScalarE
    # exp() with immediate scale; the rest drain via DVE Schraudolph
    nact = []
    for gi, g in enumerate(group_order):
        group = units[NCORES * g : NCORES * (g + 1)]
        full = min(int(L[b]) // 128 for _, b, _ in group)
        t = trips[gi]
        na = min(full, t)
        if na == t:  # keep at least one DVE tile per slot for drain balance
            na = max(0, t - max(1, round(t * 0.30)))
        nact.append(na)
    nact = tuple(nact)

    key = (trips, nact)
    if key not in _build_cache:
        _build_cache[key] = _build(trips, nact)
    nc = _build_cache[key]

    qh = q.astype(np.float16)
    kh = k.astype(np.float16)
    vh = v.astype(np.float16)

    # Schraudolph scale/bias per (key-tile, partition): for valid keys
    #   t16 = S*(SCALE*log2e*1024) + (15-C)*1024 ; int16(t16) bitcast fp16
    # masked keys get scale=bias=0 -> +0.0 exactly.
    kidx = np.arange(K)
    sc2_all = np.zeros((B, 128, KT), np.float32)
    bi2_all = np.zeros((B, 128, KT), np.float32)
    svals = np.float32(SCALE * LOG2E * 1024.0)
    bvals = np.float32((15.0 - SCHC) * 1024.0)
    for b in range(B):
        lb = int(L[b])
        if lb == 0:
            continue
        m = (kidx < lb).astype(np.float32)
        sc2_all[b] = (m * svals).reshape(KT, 128).T
        bi2_all[b] = (m * bvals).reshape(KT, 128).T

    in_maps = []
    core_units = []  # [core][slot] -> (b, h)
    for c in range(NCORES):
        slots = [units[NCORES * g + c] for g in group_order]
        core_units.append([(b, h) for _, b, h in slots])
        im = {}
        scbi = np.empty((NSLOT, 128, 2 * KT), np.float32)
        for s, (_, b, h) in enumerate(slots):
            t = trips[s]
            pack = np.empty((128, QCH + 2 * 128 * t), np.float16)
            pack[:, :QCH] = qh[b, h * QCH : (h + 1) * QCH].T
            pack[:, QCH : QCH + 128 * t] = kh[b, : 128 * t].T
            # v permuted: partition = key-within-tile, cols = (tile, d)
            pack[:, QCH + 128 * t :] = (
                vh[b, : 128 * t].reshape(t, 128, D).transpose(1, 0, 2).reshape(128, -1)
            )
            im[f"inb{s}"] = np.ascontiguousarray(pack)
            scbi[s, :, :KT] = sc2_all[b]
            scbi[s, :, KT:] = bi2_all[b]
        im["scbi"] = scbi
        in_maps.append(im)

    res = run_bass_kernel_spmd(nc, in_maps, list(range(NCORES)))
    global last_result
    last_result = res

    out = np.empty((B, Q, D), np.float32)
    for c in range(NCORES):
        r = res.results[c]
        for s in range(NSLOT):
            b, h = core_units[c][s]
            if L[b] == 0:
                out[b, h * QCH : (h + 1) * QCH] = v[b].mean(axis=0)[None, :]
                continue
            sums = r["esum"][s].astype(np.float32).sum(axis=0)  # [QCH]
            out[b, h * QCH : (h + 1) * QCH] = (
                r["oT"][s].astype(np.float32) / sums[None, :]
            ).T
    return out
